# revision 29
# baseline (speedup 1.0000x reference)
"""Channel self-attention kernel for TRN2, data-parallel over batch on 8 cores.

Math per batch element (N=4096 tokens, C=64 channels):
    q = x.reshape(N, C);  S = q @ q.T  (symmetric)
    attn = softmax(S, axis=-1);  out = gamma * (attn @ q) + x

Implementation notes (v2 — ACT+DVE split exponentials):
  - Stable softmax without online max: shift logits by t_n = ||q_n||^2 / 2.
    S_nm - t_n <= max_m |q_m|^2 / 2 (~58 for this data), so exp never
    overflows fp32/bf16, and the diagonal keeps the denominator >= 1.
    The shift is folded into the QK^T matmul as an extra contraction row:
    lhsT = [qT; 1] (65 x 128), rhs = [qT; -t], so S' = S - t_n comes out of
    the PE directly.
  - The exp is the ACT-engine wall (~1 col/cycle @1.2GHz = 135us for all
    16.7M elements).  We split it: for each row-chunk m, columns [0:512)
    are exp'd on ACT (exact, bf16 out) and columns [512:1024) on DVE via a
    Schraudolph bit-trick: with y = A*S' (A = 128/ln2, folded into the
    matmul by scaling the rhs columns), bits = max(y + B, 0) -> int16,
    bitcast to bf16 gives e^{S'} with ~2% sawtooth error.  Softmax
    renormalization cancels multiplicative weight noise (the data is
    diagonal-dominant), so the end-to-end error is unchanged (5e-4).
  - S is symmetric, so the exp'd tile Z[m, n] (keys on partitions) is the
    moving operand of the second matmul G[c, n] = sum_m vhat[m, c] Z[m, n],
    vhat = [gamma*q, 1]; G[64, n] is the softmax denominator.
  - The G matmuls trail the S matmuls by 2 iterations so the PE never
    waits on an exp; PE runs S,S,G,G back-to-back which also keeps the
    Tensor-engine p-state at full clock.
  - Prologue work is spread over gpsimd/ACT/DVE so the steady-state DVE
    exp stream is not delayed; epilogue divides run on ACT, adds on
    gpsimd, output DMA on sync.
"""
import sys
if "/opt/trn_rl_repo" not in sys.path:
    sys.path.insert(0, "/opt/trn_rl_repo")

import math
from contextlib import ExitStack

import numpy as np

import concourse.bass as bass
import concourse.mybir as mybir
import concourse.tile as tile
from concourse import bacc
from concourse.masks import make_identity

P = 128          # partitions
C = 64           # channels (head dim)
B = 8            # batch = number of cores

dt = mybir.dt
AF = mybir.ActivationFunctionType
ALU = mybir.AluOpType

LDW_OPT = False

SCH_A = 128.0 / math.log(2.0)      # Schraudolph scale (bf16: 2^7/ln2)
SCH_B = 127.0 * 128.0 - 7.0        # bias, calibrated for truncating convert


def _patch_ldw_opt():
    import concourse.bass_utils as bu
    if getattr(bu, "_ldw_opt_patch", False):
        return
    orig = bu.bir_verify_and_optimise

    def patched(*a, **kw):
        orig_run = bu.run_command

        def run2(argv, **k):
            argv = ["--enable-ldw-opt=true" if x == "--enable-ldw-opt=false" else x
                    for x in argv]
            return orig_run(argv, **k)

        bu.run_command = run2
        try:
            return orig(*a, **kw)
        finally:
            bu.run_command = orig_run

    bu.bir_verify_and_optimise = patched
    bu._ldw_opt_patch = True


def build(ntok=4096, supw=1024, z_bufs=3, s_bufs=4, pgrp=4, act_head=2,
          glag=2, warm_mm=0):
    """Build the per-core Bass module."""
    nch = ntok // P           # query/key chunks of 128
    nsup = ntok // supw       # outer n-blocks
    mw = 512                  # matmul moving width
    nmm = supw // mw          # matmuls per n-super (2)
    ech = supw // P           # epilogue 128-chunks per n-super
    pgrp = min(pgrp, nch)
    ngrp = nch // pgrp
    niter = nsup * nch

    nc = bacc.Bacc("TRN2", target_bir_lowering=False, debug=False,
                   enable_asserts=False)
    x = nc.dram_tensor("x", [ntok, C], dt.float32, kind="ExternalInput")
    g = nc.dram_tensor("gamma", [1], dt.float32, kind="ExternalInput")
    o = nc.dram_tensor("out", [ntok, C], dt.float32, kind="ExternalOutput")

    with tile.TileContext(nc) as tc, ExitStack() as ctx:
        sing = ctx.enter_context(tc.tile_pool(name="sing", bufs=1))

        ident = sing.tile([P, P], dt.float32)
        make_identity(nc, ident)
        gam = sing.tile([P, 1], dt.float32)

        # q_sb[p, k, 0:64] = x[token 32p+k, :];  q_sb[p, k, 64] = -||q||^2/2
        q_sb = sing.tile([P, nch, C + 1], dt.float32)
        # vhat[p, k, 0:64] = gamma * q, vhat[p, k, 64] = 1
        vhat = sing.tile([P, nch, C + 1], dt.bfloat16)
        ones = sing.tile([P, nch], dt.float32)
        nc.gpsimd.memset(ones, 1.0)
        sdt = dt.float16
        idh = sing.tile([P, P], sdt)
        make_identity(nc, idh)
        # qT1 = [qT; ones] (lhsT), qTt = [qT; -t] (rhs), qTtA = A*[qT; -t]
        qT1 = sing.tile([C + 1, ntok], sdt)
        qTt = sing.tile([C + 1, ntok], sdt)
        qTtA = sing.tile([C + 1, ntok], sdt)
        # preload the ACT activation table (Exp/Copy set) off the critical
        # path: the first real activation otherwise eats a ~1.3us table load.
        actw = sing.tile([P, 1], dt.float32)
        nc.vector.memset(actw, 0.0)
        nc.scalar.activation(out=actw, in_=actw, func=AF.Exp)

        # permuted token order: partition p holds tokens 32p..32p+31 so each
        # partition reads one contiguous 8KB run of x.
        xg = x.ap().rearrange("(p k) c -> p k c", k=nch)
        og = o.ap().rearrange("(p k) c -> p k c", k=nch)
        sqp = ctx.enter_context(tc.tile_pool(name="sqp", bufs=2))
        aux = ctx.enter_context(tc.tile_pool(name="aux", bufs=2, space="PSUM"))
        spool = ctx.enter_context(tc.tile_pool(name="spool", bufs=s_bufs, space="PSUM"))
        gpool = ctx.enter_context(tc.tile_pool(name="gpool", bufs=1, space="PSUM"))
        zpool = ctx.enter_context(tc.tile_pool(name="zpool", bufs=z_bufs))
        gsb = ctx.enter_context(tc.tile_pool(name="gsb", bufs=2))
        esb = ctx.enter_context(tc.tile_pool(name="esb", bufs=4))

        # group layout: two 2-chunk groups first (shorter critical path to
        # the first matmul), then pgrp-wide groups.
        groups = []
        _head = [2, 2, 4] if pgrp > 4 else [2, 2]
        _c = 0
        while _c < nch:
            gi = len(groups)
            sz = _head[gi] if gi < len(_head) else pgrp
            sz = min(sz, nch - _c)
            groups.append((_c, sz))
            _c += sz
        ngrp = len(groups)

        # issue every input DMA upfront on alternating queues; transfers
        # overlap the framework init and each other.
        for gi, (cg, csz) in enumerate(groups):
            ks = slice(cg, cg + csz)
            (nc.sync if gi % 2 == 0 else nc.gpsimd).dma_start(
                out=q_sb[:, ks, 0:C], in_=xg[:, ks, :])
            if gi == 0:
                nc.gpsimd.dma_start(out=gam, in_=g.ap().to_broadcast((P, 1)))

        # dummy matmuls ramp the Tensor-engine p-state (full clock needs
        # ~3us of continuous busy) and bridge the PE until the first real
        # transposes/matmuls are ready (~14us: dma + norms + casts).
        if warm_mm:
            dmr = sing.tile([P, mw], sdt)
            nc.vector.memset(dmr, 1.0)
            for _ in range(warm_mm):
                wt = aux.tile([P, mw], dt.float32, tag="aux", name="wt")
                nc.tensor.matmul(wt, idh, dmr, start=True, stop=True)

        def emit_group(gi):
            """Preprocess one chunk group (its DMA was issued upfront).

            Engine split keeps every step on an engine that is fast at it
            (gpsimd only gets the one big elementwise mul; its tensor_scalar
            and semaphore handling are ~1-4us each and poison the pipeline).
            """
            cg, csz = groups[gi]
            ks = slice(cg, cg + csz)
            sq = sqp.tile([P, csz, C], dt.float32)
            (nc.vector if gi < 2 else nc.gpsimd).tensor_mul(
                sq, q_sb[:, ks, 0:C], q_sb[:, ks, 0:C])
            rg = sqp.tile([P, csz], dt.float32, tag="rg")
            nc.vector.reduce_sum(out=rg, in_=sq, axis=mybir.AxisListType.X)
            nc.vector.tensor_scalar_mul(q_sb[:, ks, C : C + 1],
                                        rg.unsqueeze(2), -0.5)
            qf = sqp.tile([P, csz, C + 1], sdt, tag="qf")
            nc.scalar.copy(out=qf, in_=q_sb[:, ks, :])
            tp = aux.tile([C + 1, csz * P], sdt, tag="aux")
            for j in range(csz):
                nc.tensor.transpose(out=tp[:, j * P : (j + 1) * P],
                                    in_=qf[:, j, :], identity=idh)
            sl = slice(cg * P, (cg + csz) * P)
            nc.vector.tensor_copy(out=qTt[:, sl], in_=tp)
            nc.scalar.mul(qTtA[:, sl], tp, SCH_A)
            nc.vector.tensor_copy(out=qT1[0:C, sl], in_=tp[0:C, :])
            # ones row of the lhsT, split per group: a single [1, ntok]
            # memset is a ~4us single-partition op that clogs its engine.
            nc.gpsimd.memset(qT1[C : C + 1, sl], 1.0)
            # vhat emitted after the critical transpose->copy chain
            nc.scalar.activation(out=vhat[:, ks, 0:C], in_=q_sb[:, ks, 0:C],
                                 func=AF.Copy, scale=gam)
            nc.vector.tensor_copy(out=vhat[:, ks, C : C + 1],
                                  in_=ones[:, ks].unsqueeze(2))

        emitted = 0      # groups emitted
        covered = 0      # chunks covered by emitted groups

        def need_chunks(n):
            nonlocal emitted, covered
            while covered < min(n, nch):
                emit_group(emitted)
                covered += groups[emitted][1]
                emitted += 1

        gts = {}      # js -> (gt0, gt1) gpool half-tiles
        zts = {}      # it -> zpool tile

        def emit_G(it):
            js2, m2 = divmod(it, nch)
            if m2 == 0:
                gt = gpool.tile([C + 1, supw], dt.float32, tag="gt")
                gts[js2] = (gt[:, 0:mw], gt[:, mw : 2 * mw])
            gth = gts[js2]
            zb = zts.pop(it).bitcast(dt.bfloat16)
            for i in range(nmm):
                nc.tensor.matmul(gth[i], vhat[:, m2, :],
                                 zb[:, i * mw : (i + 1) * mw],
                                 start=(m2 == 0), stop=(m2 == nch - 1))
            if m2 == nch - 1:
                emit_epilogue(js2, gth)

        epi_queue = []    # deferred per-chunk-pair epilogue work items

        def emit_epilogue(js2, gth):
            # gt -> SBUF copies stay immediate (they free the gpool bank);
            # per-chunk work is deferred and interleaved into following
            # iterations so it doesn't stall the pipeline in a burst.
            gs = gsb.tile([C + 1, supw], dt.float32)
            nc.scalar.copy(out=gs[:, 0:mw], in_=gth[0])
            nc.vector.tensor_copy(out=gs[:, mw : 2 * mw], in_=gth[1])
            for e in range(0, ech, 2):
                epi_queue.append((js2, e, gs))

        def emit_epi_pair(js2, e, gs):
            """Finish chunks e, e+1 of super js2: transpose, divide by the
            denominator, add the residual, store (2 chunks per DMA)."""
            oc = esb.tile([P, 2, C], dt.float32, tag="oc")
            for k in range(2):
                ch = js2 * ech + e + k
                gtp = aux.tile([P, C + 1], dt.float32, tag="aux")
                nc.tensor.transpose(out=gtp,
                                    in_=gs[:, (e + k) * P : (e + k + 1) * P],
                                    identity=ident[0 : C + 1, 0 : C + 1])
                rec = esb.tile([P, 1], dt.float32)
                nc.vector.reciprocal(out=rec, in_=gtp[:, C : C + 1])
                nc.vector.affine_then_add(out=oc[:, k, :], in0=gtp[:, 0:C],
                                          in1=q_sb[:, ch, 0:C],
                                          scale=rec, bias=0.0)
            ch0 = js2 * ech + e
            (nc.sync if (e // 2) % 2 == 0 else nc.gpsimd).dma_start(
                out=og[:, ch0 : ch0 + 2, :], in_=oc)

        for it in range(niter):
            js, m = divmod(it, nch)
            # prefetch prologue chunks: the rhs needs all chunks covering the
            # super's columns; the lhsT needs chunk m (+ lookahead).
            need_chunks(max((js + 1) * (supw // P), m + 6))
            act_full = it < act_head
            st0 = spool.tile([P, mw], dt.float32, tag="st")
            st1 = spool.tile([P, mw], dt.float32, tag="st")
            lhs = qT1[:, m * P : (m + 1) * P]
            c0 = js * supw
            nc.tensor.matmul(st0, lhs, qTt[:, c0 : c0 + mw],
                             start=True, stop=True)
            rhs2 = (qTt if act_full else qTtA)[:, c0 + mw : c0 + 2 * mw]
            nc.tensor.matmul(st1, lhs, rhs2, start=True, stop=True)
            zt = zpool.tile([P, supw], dt.int16)
            zts[it] = zt
            zb = zt.bitcast(dt.bfloat16)
            nc.scalar.activation(out=zb[:, 0:mw], in_=st0, func=AF.Exp)
            if act_full:
                nc.scalar.activation(out=zb[:, mw : 2 * mw], in_=st1,
                                     func=AF.Exp)
            else:
                nc.vector.tensor_scalar(out=zt[:, mw : 2 * mw], in0=st1,
                                        scalar1=SCH_B, scalar2=0.0,
                                        op0=ALU.add, op1=ALU.max)
            if it >= glag:
                emit_G(it - glag)
            if epi_queue and it % 2 == 0:
                emit_epi_pair(*epi_queue.pop(0))
        for it in range(niter - glag, niter):
            emit_G(it)
        while epi_queue:
            emit_epi_pair(*epi_queue.pop(0))

    nc.compile()
    return nc


_CACHE = {}


def _get_nc(**kw):
    key = tuple(sorted(kw.items()))
    if key not in _CACHE:
        _CACHE[key] = build(**kw)
    return _CACHE[key]


def kernel(x: np.ndarray, gamma: np.ndarray) -> np.ndarray:
    """Full-input entry point: x (8,16,16,16,64) f32, gamma (1,) f32."""
    if LDW_OPT:
        _patch_ldw_opt()
    from concourse.bass_utils import run_bass_kernel_spmd

    Bf, D, H, W, Cf = x.shape
    ntok = D * H * W
    xf = np.ascontiguousarray(np.asarray(x, dtype=np.float32).reshape(Bf, ntok, Cf))
    gf = np.ascontiguousarray(np.asarray(gamma, dtype=np.float32).reshape(1))
    nc = _get_nc(ntok=ntok)
    in_maps = [{"x": xf[b], "gamma": gf} for b in range(Bf)]
    res = run_bass_kernel_spmd(nc, in_maps, core_ids=list(range(Bf)))
    out = np.stack([res.results[b]["out"] for b in range(Bf)], axis=0)
    return out.reshape(x.shape).astype(x.dtype, copy=False)


# revision 30
# speedup vs baseline: 1.0226x; 1.0226x over previous
"""Channel self-attention kernel for TRN2, data-parallel over batch on 8 cores.

Math per batch element (N=4096 tokens, C=64 channels):
    q = x.reshape(N, C);  S = q @ q.T  (symmetric)
    attn = softmax(S, axis=-1);  out = gamma * (attn @ q) + x

Implementation notes (v2 — ACT+DVE split exponentials):
  - Stable softmax without online max: shift logits by t_n = ||q_n||^2 / 2.
    S_nm - t_n <= max_m |q_m|^2 / 2 (~58 for this data), so exp never
    overflows fp32/bf16, and the diagonal keeps the denominator >= 1.
    The shift is folded into the QK^T matmul as an extra contraction row:
    lhsT = [qT; 1] (65 x 128), rhs = [qT; -t], so S' = S - t_n comes out of
    the PE directly.
  - The exp is the ACT-engine wall (~1 col/cycle @1.2GHz = 135us for all
    16.7M elements).  We split it: for each row-chunk m, columns [0:512)
    are exp'd on ACT (exact, bf16 out) and columns [512:1024) on DVE via a
    Schraudolph bit-trick: with y = A*S' (A = 128/ln2, folded into the
    matmul by scaling the rhs columns), bits = max(y + B, 0) -> int16,
    bitcast to bf16 gives e^{S'} with ~2% sawtooth error.  Softmax
    renormalization cancels multiplicative weight noise (the data is
    diagonal-dominant), so the end-to-end error is unchanged (5e-4).
  - S is symmetric, so the exp'd tile Z[m, n] (keys on partitions) is the
    moving operand of the second matmul G[c, n] = sum_m vhat[m, c] Z[m, n],
    vhat = [gamma*q, 1]; G[64, n] is the softmax denominator.
  - The G matmuls trail the S matmuls by 2 iterations so the PE never
    waits on an exp; PE runs S,S,G,G back-to-back which also keeps the
    Tensor-engine p-state at full clock.
  - Prologue work is spread over gpsimd/ACT/DVE so the steady-state DVE
    exp stream is not delayed; epilogue divides run on ACT, adds on
    gpsimd, output DMA on sync.
"""
import sys
if "/opt/trn_rl_repo" not in sys.path:
    sys.path.insert(0, "/opt/trn_rl_repo")

import math
from contextlib import ExitStack

import numpy as np

import concourse.bass as bass
import concourse.mybir as mybir
import concourse.tile as tile
from concourse import bacc
from concourse.masks import make_identity

P = 128          # partitions
C = 64           # channels (head dim)
B = 8            # batch = number of cores

dt = mybir.dt
AF = mybir.ActivationFunctionType
ALU = mybir.AluOpType

LDW_OPT = False

SCH_A = 128.0 / math.log(2.0)      # Schraudolph scale (bf16: 2^7/ln2)
SCH_B = 127.0 * 128.0 - 7.0        # bias, calibrated for truncating convert


def _patch_ldw_opt():
    import concourse.bass_utils as bu
    if getattr(bu, "_ldw_opt_patch", False):
        return
    orig = bu.bir_verify_and_optimise

    def patched(*a, **kw):
        orig_run = bu.run_command

        def run2(argv, **k):
            argv = ["--enable-ldw-opt=true" if x == "--enable-ldw-opt=false" else x
                    for x in argv]
            return orig_run(argv, **k)

        bu.run_command = run2
        try:
            return orig(*a, **kw)
        finally:
            bu.run_command = orig_run

    bu.bir_verify_and_optimise = patched
    bu._ldw_opt_patch = True


def build(ntok=4096, supw=1024, z_bufs=3, s_bufs=4, pgrp=4, act_head=2,
          glag=2, warm_mm=0):
    """Build the per-core Bass module."""
    nch = ntok // P           # query/key chunks of 128
    nsup = ntok // supw       # outer n-blocks
    mw = 512                  # matmul moving width
    nmm = supw // mw          # matmuls per n-super (2)
    ech = supw // P           # epilogue 128-chunks per n-super
    pgrp = min(pgrp, nch)
    ngrp = nch // pgrp
    niter = nsup * nch

    nc = bacc.Bacc("TRN2", target_bir_lowering=False, debug=False,
                   enable_asserts=False)
    x = nc.dram_tensor("x", [ntok, C], dt.float32, kind="ExternalInput")
    g = nc.dram_tensor("gamma", [1], dt.float32, kind="ExternalInput")
    o = nc.dram_tensor("out", [ntok, C], dt.float32, kind="ExternalOutput")

    with tile.TileContext(nc) as tc, ExitStack() as ctx:
        sing = ctx.enter_context(tc.tile_pool(name="sing", bufs=1))

        ident = sing.tile([P, P], dt.float32)
        make_identity(nc, ident)
        gam = sing.tile([P, 1], dt.float32)

        # q_sb[p, k, 0:64] = x[token 32p+k, :];  q_sb[p, k, 64] = -||q||^2/2
        q_sb = sing.tile([P, nch, C + 1], dt.float32)
        # vhat[p, k, 0:64] = gamma * q, vhat[p, k, 64] = 1
        vhat = sing.tile([P, nch, C + 1], dt.bfloat16)
        ones = sing.tile([P, nch], dt.float32)
        nc.gpsimd.memset(ones, 1.0)
        sdt = dt.float16
        idh = sing.tile([P, P], sdt)
        make_identity(nc, idh)
        # qT1 = [qT; ones] (lhsT), qTt = [qT; -t] (rhs), qTtA = A*[qT; -t]
        qT1 = sing.tile([C + 1, ntok], sdt)
        qTt = sing.tile([C + 1, ntok], sdt)
        qTtA = sing.tile([C + 1, ntok], sdt)
        # preload the ACT activation table (Exp/Copy set) off the critical
        # path: the first real activation otherwise eats a ~1.3us table load.
        actw = sing.tile([P, 1], dt.float32)
        nc.vector.memset(actw, 0.0)
        nc.scalar.activation(out=actw, in_=actw, func=AF.Exp)

        # permuted token order: partition p holds tokens 32p..32p+31 so each
        # partition reads one contiguous 8KB run of x.
        xg = x.ap().rearrange("(p k) c -> p k c", k=nch)
        og = o.ap().rearrange("(p k) c -> p k c", k=nch)
        sqp = ctx.enter_context(tc.tile_pool(name="sqp", bufs=2))
        aux = ctx.enter_context(tc.tile_pool(name="aux", bufs=2, space="PSUM"))
        spool = ctx.enter_context(tc.tile_pool(name="spool", bufs=s_bufs, space="PSUM"))
        gpool = ctx.enter_context(tc.tile_pool(name="gpool", bufs=1, space="PSUM"))
        zpool = ctx.enter_context(tc.tile_pool(name="zpool", bufs=z_bufs))
        gsb = ctx.enter_context(tc.tile_pool(name="gsb", bufs=2))
        esb = ctx.enter_context(tc.tile_pool(name="esb", bufs=4))

        # group layout: two 2-chunk groups first (shorter critical path to
        # the first matmul), then pgrp-wide groups.
        groups = []
        _head = [2, 2, 4] if pgrp > 4 else [2, 2]
        _c = 0
        while _c < nch:
            gi = len(groups)
            sz = _head[gi] if gi < len(_head) else pgrp
            sz = min(sz, nch - _c)
            groups.append((_c, sz))
            _c += sz
        ngrp = len(groups)

        # issue every input DMA upfront on alternating queues; transfers
        # overlap the framework init and each other.
        for gi, (cg, csz) in enumerate(groups):
            ks = slice(cg, cg + csz)
            (nc.sync if gi % 2 == 0 else nc.gpsimd).dma_start(
                out=q_sb[:, ks, 0:C], in_=xg[:, ks, :])
            if gi == 0:
                nc.gpsimd.dma_start(out=gam, in_=g.ap().to_broadcast((P, 1)))

        # dummy matmuls ramp the Tensor-engine p-state (full clock needs
        # ~3us of continuous busy) and bridge the PE until the first real
        # transposes/matmuls are ready (~14us: dma + norms + casts).
        if warm_mm:
            dmr = sing.tile([P, mw], sdt)
            nc.vector.memset(dmr, 1.0)
            for _ in range(warm_mm):
                wt = aux.tile([P, mw], dt.float32, tag="aux", name="wt")
                nc.tensor.matmul(wt, idh, dmr, start=True, stop=True)

        def emit_group(gi):
            """Preprocess one chunk group (its DMA was issued upfront).

            Engine split keeps every step on an engine that is fast at it
            (gpsimd only gets the one big elementwise mul; its tensor_scalar
            and semaphore handling are ~1-4us each and poison the pipeline).
            """
            cg, csz = groups[gi]
            ks = slice(cg, cg + csz)
            sq = sqp.tile([P, csz, C], dt.float32)
            (nc.vector if gi < 2 else nc.gpsimd).tensor_mul(
                sq, q_sb[:, ks, 0:C], q_sb[:, ks, 0:C])
            rg = sqp.tile([P, csz], dt.float32, tag="rg")
            nc.vector.reduce_sum(out=rg, in_=sq, axis=mybir.AxisListType.X)
            nc.vector.tensor_scalar_mul(q_sb[:, ks, C : C + 1],
                                        rg.unsqueeze(2), -0.5)
            qf = sqp.tile([P, csz, C + 1], sdt, tag="qf")
            nc.scalar.copy(out=qf, in_=q_sb[:, ks, :])
            tp = aux.tile([C + 1, csz * P], sdt, tag="aux")
            for j in range(csz):
                nc.tensor.transpose(out=tp[:, j * P : (j + 1) * P],
                                    in_=qf[:, j, :], identity=idh)
            sl = slice(cg * P, (cg + csz) * P)
            nc.vector.tensor_copy(out=qTt[:, sl], in_=tp)
            nc.scalar.mul(qTtA[:, sl], tp, SCH_A)
            nc.vector.tensor_copy(out=qT1[0:C, sl], in_=tp[0:C, :])
            # ones row of the lhsT, split per group: a single [1, ntok]
            # memset is a ~4us single-partition op that clogs its engine.
            nc.gpsimd.memset(qT1[C : C + 1, sl], 1.0)
            # vhat emitted after the critical transpose->copy chain
            nc.scalar.activation(out=vhat[:, ks, 0:C], in_=q_sb[:, ks, 0:C],
                                 func=AF.Copy, scale=gam)
            nc.vector.tensor_copy(out=vhat[:, ks, C : C + 1],
                                  in_=ones[:, ks].unsqueeze(2))

        emitted = 0      # groups emitted
        covered = 0      # chunks covered by emitted groups

        def need_chunks(n):
            nonlocal emitted, covered
            while covered < min(n, nch):
                emit_group(emitted)
                covered += groups[emitted][1]
                emitted += 1

        gts = {}      # js -> (gt0, gt1) gpool half-tiles
        zts = {}      # it -> zpool tile

        def emit_G(it):
            js2, m2 = divmod(it, nch)
            if m2 == 0:
                gt = gpool.tile([C + 1, supw], dt.float32, tag="gt")
                gts[js2] = (gt[:, 0:mw], gt[:, mw : 2 * mw])
            gth = gts[js2]
            zb = zts.pop(it).bitcast(dt.bfloat16)
            for i in range(nmm):
                nc.tensor.matmul(gth[i], vhat[:, m2, :],
                                 zb[:, i * mw : (i + 1) * mw],
                                 start=(m2 == 0), stop=(m2 == nch - 1))
            if m2 == nch - 1:
                emit_epilogue(js2, gth)

        epi_queue = []    # deferred per-chunk-pair epilogue work items

        def emit_epilogue(js2, gth):
            # gt -> SBUF copies stay immediate (they free the gpool bank);
            # per-chunk work is deferred and interleaved into following
            # iterations so it doesn't stall the pipeline in a burst.
            gs = gsb.tile([C + 1, supw], dt.float32)
            nc.scalar.copy(out=gs[:, 0:mw], in_=gth[0])
            nc.vector.tensor_copy(out=gs[:, mw : 2 * mw], in_=gth[1])
            for e in range(0, ech, 2):
                epi_queue.append((js2, e, gs))

        def emit_epi_pair(js2, e, gs):
            """Finish chunks e, e+1 of super js2: transpose, divide by the
            denominator, add the residual, store (2 chunks per DMA)."""
            oc = esb.tile([P, 2, C], dt.float32, tag="oc")
            for k in range(2):
                ch = js2 * ech + e + k
                gtp = aux.tile([P, C + 1], dt.float32, tag="aux")
                nc.tensor.transpose(out=gtp,
                                    in_=gs[:, (e + k) * P : (e + k + 1) * P],
                                    identity=ident[0 : C + 1, 0 : C + 1])
                rec = esb.tile([P, 1], dt.float32)
                nc.vector.reciprocal(out=rec, in_=gtp[:, C : C + 1])
                nc.vector.affine_then_add(out=oc[:, k, :], in0=gtp[:, 0:C],
                                          in1=q_sb[:, ch, 0:C],
                                          scale=rec, bias=0.0)
            ch0 = js2 * ech + e
            # the last super's stores must all ride sync: a store queued on
            # gpsimd behind its (slow, ~2.3us) teardown drain executes ~7us
            # late and stretches the measured kernel end.
            last = js2 == nsup - 1
            (nc.sync if last or (e // 2) % 2 == 0 else nc.gpsimd).dma_start(
                out=og[:, ch0 : ch0 + 2, :], in_=oc)

        for it in range(niter):
            js, m = divmod(it, nch)
            # prefetch prologue chunks: the rhs needs all chunks covering the
            # super's columns; the lhsT needs chunk m (+ lookahead).
            need_chunks(max((js + 1) * (supw // P), m + 6))
            act_full = it < act_head
            st0 = spool.tile([P, mw], dt.float32, tag="st")
            st1 = spool.tile([P, mw], dt.float32, tag="st")
            lhs = qT1[:, m * P : (m + 1) * P]
            c0 = js * supw
            nc.tensor.matmul(st0, lhs, qTt[:, c0 : c0 + mw],
                             start=True, stop=True)
            rhs2 = (qTt if act_full else qTtA)[:, c0 + mw : c0 + 2 * mw]
            nc.tensor.matmul(st1, lhs, rhs2, start=True, stop=True)
            zt = zpool.tile([P, supw], dt.int16)
            zts[it] = zt
            zb = zt.bitcast(dt.bfloat16)
            nc.scalar.activation(out=zb[:, 0:mw], in_=st0, func=AF.Exp)
            if act_full:
                nc.scalar.activation(out=zb[:, mw : 2 * mw], in_=st1,
                                     func=AF.Exp)
            else:
                nc.vector.tensor_scalar(out=zt[:, mw : 2 * mw], in0=st1,
                                        scalar1=SCH_B, scalar2=0.0,
                                        op0=ALU.add, op1=ALU.max)
            if it >= glag:
                emit_G(it - glag)
            if epi_queue and it % 2 == 0:
                emit_epi_pair(*epi_queue.pop(0))
        for it in range(niter - glag, niter):
            emit_G(it)
        while epi_queue:
            emit_epi_pair(*epi_queue.pop(0))

    nc.compile()
    return nc


_CACHE = {}


def _get_nc(**kw):
    key = tuple(sorted(kw.items()))
    if key not in _CACHE:
        _CACHE[key] = build(**kw)
    return _CACHE[key]


def kernel(x: np.ndarray, gamma: np.ndarray) -> np.ndarray:
    """Full-input entry point: x (8,16,16,16,64) f32, gamma (1,) f32."""
    if LDW_OPT:
        _patch_ldw_opt()
    from concourse.bass_utils import run_bass_kernel_spmd

    Bf, D, H, W, Cf = x.shape
    ntok = D * H * W
    xf = np.ascontiguousarray(np.asarray(x, dtype=np.float32).reshape(Bf, ntok, Cf))
    gf = np.ascontiguousarray(np.asarray(gamma, dtype=np.float32).reshape(1))
    nc = _get_nc(ntok=ntok)
    in_maps = [{"x": xf[b], "gamma": gf} for b in range(Bf)]
    res = run_bass_kernel_spmd(nc, in_maps, core_ids=list(range(Bf)))
    out = np.stack([res.results[b]["out"] for b in range(Bf)], axis=0)
    return out.reshape(x.shape).astype(x.dtype, copy=False)


# revision 31
# speedup vs baseline: 1.0399x; 1.0169x over previous
"""Channel self-attention kernel for TRN2, data-parallel over batch on 8 cores.

Math per batch element (N=4096 tokens, C=64 channels):
    q = x.reshape(N, C);  S = q @ q.T  (symmetric)
    attn = softmax(S, axis=-1);  out = gamma * (attn @ q) + x

Implementation notes (v2 — ACT+DVE split exponentials):
  - Stable softmax without online max: shift logits by t_n = ||q_n||^2 / 2.
    S_nm - t_n <= max_m |q_m|^2 / 2 (~58 for this data), so exp never
    overflows fp32/bf16, and the diagonal keeps the denominator >= 1.
    The shift is folded into the QK^T matmul as an extra contraction row:
    lhsT = [qT; 1] (65 x 128), rhs = [qT; -t], so S' = S - t_n comes out of
    the PE directly.
  - The exp is the ACT-engine wall (~1 col/cycle @1.2GHz = 135us for all
    16.7M elements).  We split it: for each row-chunk m, columns [0:512)
    are exp'd on ACT (exact, bf16 out) and columns [512:1024) on DVE via a
    Schraudolph bit-trick: with y = A*S' (A = 128/ln2, folded into the
    matmul by scaling the rhs columns), bits = max(y + B, 0) -> int16,
    bitcast to bf16 gives e^{S'} with ~2% sawtooth error.  Softmax
    renormalization cancels multiplicative weight noise (the data is
    diagonal-dominant), so the end-to-end error is unchanged (5e-4).
  - S is symmetric, so the exp'd tile Z[m, n] (keys on partitions) is the
    moving operand of the second matmul G[c, n] = sum_m vhat[m, c] Z[m, n],
    vhat = [gamma*q, 1]; G[64, n] is the softmax denominator.
  - The G matmuls trail the S matmuls by 2 iterations so the PE never
    waits on an exp; PE runs S,S,G,G back-to-back which also keeps the
    Tensor-engine p-state at full clock.
  - Prologue work is spread over gpsimd/ACT/DVE so the steady-state DVE
    exp stream is not delayed; epilogue divides run on ACT, adds on
    gpsimd, output DMA on sync.
"""
import sys
if "/opt/trn_rl_repo" not in sys.path:
    sys.path.insert(0, "/opt/trn_rl_repo")

import math
from contextlib import ExitStack

import numpy as np

import concourse.bass as bass
import concourse.mybir as mybir
import concourse.tile as tile
from concourse import bacc
from concourse.masks import make_identity

P = 128          # partitions
C = 64           # channels (head dim)
B = 8            # batch = number of cores

dt = mybir.dt
AF = mybir.ActivationFunctionType
ALU = mybir.AluOpType

LDW_OPT = False

SCH_A = 128.0 / math.log(2.0)      # Schraudolph scale (bf16: 2^7/ln2)
SCH_B = 127.0 * 128.0 - 7.0        # bias, calibrated for truncating convert


def _patch_ldw_opt():
    import concourse.bass_utils as bu
    if getattr(bu, "_ldw_opt_patch", False):
        return
    orig = bu.bir_verify_and_optimise

    def patched(*a, **kw):
        orig_run = bu.run_command

        def run2(argv, **k):
            argv = ["--enable-ldw-opt=true" if x == "--enable-ldw-opt=false" else x
                    for x in argv]
            return orig_run(argv, **k)

        bu.run_command = run2
        try:
            return orig(*a, **kw)
        finally:
            bu.run_command = orig_run

    bu.bir_verify_and_optimise = patched
    bu._ldw_opt_patch = True


def build(ntok=4096, supw=1024, z_bufs=3, s_bufs=4, pgrp=8, act_head=2,
          glag=2, warm_mm=0):
    """Build the per-core Bass module."""
    nch = ntok // P           # query/key chunks of 128
    nsup = ntok // supw       # outer n-blocks
    mw = 512                  # matmul moving width
    nmm = supw // mw          # matmuls per n-super (2)
    ech = supw // P           # epilogue 128-chunks per n-super
    pgrp = min(pgrp, nch)
    ngrp = nch // pgrp
    niter = nsup * nch

    nc = bacc.Bacc("TRN2", target_bir_lowering=False, debug=False,
                   enable_asserts=False)
    x = nc.dram_tensor("x", [ntok, C], dt.float32, kind="ExternalInput")
    g = nc.dram_tensor("gamma", [1], dt.float32, kind="ExternalInput")
    o = nc.dram_tensor("out", [ntok, C], dt.float32, kind="ExternalOutput")

    with tile.TileContext(nc) as tc, ExitStack() as ctx:
        sing = ctx.enter_context(tc.tile_pool(name="sing", bufs=1))

        ident = sing.tile([P, P], dt.float32)
        make_identity(nc, ident)
        gam = sing.tile([P, 1], dt.float32)

        # q_sb[p, k, 0:64] = x[token 32p+k, :];  q_sb[p, k, 64] = -||q||^2/2
        q_sb = sing.tile([P, nch, C + 1], dt.float32)
        # vhat[p, k, 0:64] = gamma * q, vhat[p, k, 64] = 1
        vhat = sing.tile([P, nch, C + 1], dt.bfloat16)
        ones = sing.tile([P, nch], dt.float32)
        nc.gpsimd.memset(ones, 1.0)
        sdt = dt.float16
        idh = sing.tile([P, P], sdt)
        make_identity(nc, idh)
        # qT1 = [qT; ones] (lhsT), qTt = [qT; -t] (rhs), qTtA = A*[qT; -t]
        qT1 = sing.tile([C + 1, ntok], sdt)
        qTt = sing.tile([C + 1, ntok], sdt)
        qTtA = sing.tile([C + 1, ntok], sdt)
        # preload the ACT activation table (Exp/Copy set) off the critical
        # path: the first real activation otherwise eats a ~1.3us table load.
        actw = sing.tile([P, 1], dt.float32)
        nc.vector.memset(actw, 0.0)
        nc.scalar.activation(out=actw, in_=actw, func=AF.Exp)

        # permuted token order: partition p holds tokens 32p..32p+31 so each
        # partition reads one contiguous 8KB run of x.
        xg = x.ap().rearrange("(p k) c -> p k c", k=nch)
        og = o.ap().rearrange("(p k) c -> p k c", k=nch)
        sqp = ctx.enter_context(tc.tile_pool(name="sqp", bufs=2))
        aux = ctx.enter_context(tc.tile_pool(name="aux", bufs=2, space="PSUM"))
        spool = ctx.enter_context(tc.tile_pool(name="spool", bufs=s_bufs, space="PSUM"))
        gpool = ctx.enter_context(tc.tile_pool(name="gpool", bufs=1, space="PSUM"))
        zpool = ctx.enter_context(tc.tile_pool(name="zpool", bufs=z_bufs))
        gsb = ctx.enter_context(tc.tile_pool(name="gsb", bufs=2))
        esb = ctx.enter_context(tc.tile_pool(name="esb", bufs=4))

        # group layout: two 2-chunk groups first (shorter critical path to
        # the first matmul), then pgrp-wide groups.
        groups = []
        _head = [2, 2, 4] if pgrp > 4 else [2, 2]
        _c = 0
        while _c < nch:
            gi = len(groups)
            sz = _head[gi] if gi < len(_head) else pgrp
            sz = min(sz, nch - _c)
            groups.append((_c, sz))
            _c += sz
        ngrp = len(groups)

        # issue every input DMA upfront on alternating queues; transfers
        # overlap the framework init and each other.
        for gi, (cg, csz) in enumerate(groups):
            ks = slice(cg, cg + csz)
            (nc.sync if gi % 2 == 0 else nc.gpsimd).dma_start(
                out=q_sb[:, ks, 0:C], in_=xg[:, ks, :])
            if gi == 0:
                nc.gpsimd.dma_start(out=gam, in_=g.ap().to_broadcast((P, 1)))

        # dummy matmuls ramp the Tensor-engine p-state (full clock needs
        # ~3us of continuous busy) and bridge the PE until the first real
        # transposes/matmuls are ready (~14us: dma + norms + casts).
        if warm_mm:
            dmr = sing.tile([P, mw], sdt)
            nc.vector.memset(dmr, 1.0)
            for _ in range(warm_mm):
                wt = aux.tile([P, mw], dt.float32, tag="aux", name="wt")
                nc.tensor.matmul(wt, idh, dmr, start=True, stop=True)

        def emit_group(gi):
            """Preprocess one chunk group (its DMA was issued upfront).

            Engine split keeps every step on an engine that is fast at it
            (gpsimd only gets the one big elementwise mul; its tensor_scalar
            and semaphore handling are ~1-4us each and poison the pipeline).
            """
            cg, csz = groups[gi]
            ks = slice(cg, cg + csz)
            sq = sqp.tile([P, csz, C], dt.float32)
            (nc.vector if gi < 2 else nc.gpsimd).tensor_mul(
                sq, q_sb[:, ks, 0:C], q_sb[:, ks, 0:C])
            rg = sqp.tile([P, csz], dt.float32, tag="rg")
            nc.vector.reduce_sum(out=rg, in_=sq, axis=mybir.AxisListType.X)
            nc.vector.tensor_scalar_mul(q_sb[:, ks, C : C + 1],
                                        rg.unsqueeze(2), -0.5)
            qf = sqp.tile([P, csz, C + 1], sdt, tag="qf")
            nc.scalar.copy(out=qf, in_=q_sb[:, ks, :])
            tp = aux.tile([C + 1, csz * P], sdt, tag="aux")
            for j in range(csz):
                nc.tensor.transpose(out=tp[:, j * P : (j + 1) * P],
                                    in_=qf[:, j, :], identity=idh)
            sl = slice(cg * P, (cg + csz) * P)
            nc.vector.tensor_copy(out=qTt[:, sl], in_=tp)
            nc.scalar.mul(qTtA[:, sl], tp, SCH_A)
            nc.vector.tensor_copy(out=qT1[0:C, sl], in_=tp[0:C, :])
            # ones row of the lhsT, split per group: a single [1, ntok]
            # memset is a ~4us single-partition op that clogs its engine.
            nc.gpsimd.memset(qT1[C : C + 1, sl], 1.0)
            # vhat emitted after the critical transpose->copy chain
            nc.scalar.activation(out=vhat[:, ks, 0:C], in_=q_sb[:, ks, 0:C],
                                 func=AF.Copy, scale=gam)
            nc.vector.tensor_copy(out=vhat[:, ks, C : C + 1],
                                  in_=ones[:, ks].unsqueeze(2))

        emitted = 0      # groups emitted
        covered = 0      # chunks covered by emitted groups

        def need_chunks(n):
            nonlocal emitted, covered
            while covered < min(n, nch):
                emit_group(emitted)
                covered += groups[emitted][1]
                emitted += 1

        gts = {}      # js -> (gt0, gt1) gpool half-tiles
        zts = {}      # it -> zpool tile

        def emit_G(it):
            js2, m2 = divmod(it, nch)
            if m2 == 0:
                gt = gpool.tile([C + 1, supw], dt.float32, tag="gt")
                gts[js2] = (gt[:, 0:mw], gt[:, mw : 2 * mw])
            gth = gts[js2]
            zb = zts.pop(it).bitcast(dt.bfloat16)
            for i in range(nmm):
                nc.tensor.matmul(gth[i], vhat[:, m2, :],
                                 zb[:, i * mw : (i + 1) * mw],
                                 start=(m2 == 0), stop=(m2 == nch - 1))
            if m2 == nch - 1:
                emit_epilogue(js2, gth)

        epi_queue = []    # deferred per-chunk-pair epilogue work items

        def emit_epilogue(js2, gth):
            # gt -> SBUF copies stay immediate (they free the gpool bank);
            # per-chunk work is deferred and interleaved into following
            # iterations so it doesn't stall the pipeline in a burst.
            gs = gsb.tile([C + 1, supw], dt.float32)
            nc.scalar.copy(out=gs[:, 0:mw], in_=gth[0])
            nc.vector.tensor_copy(out=gs[:, mw : 2 * mw], in_=gth[1])
            for e in range(0, ech, 2):
                epi_queue.append((js2, e, gs))

        def emit_epi_pair(js2, e, gs):
            """Finish chunks e, e+1 of super js2: transpose, divide by the
            denominator, add the residual, store (2 chunks per DMA)."""
            oc = esb.tile([P, 2, C], dt.float32, tag="oc")
            for k in range(2):
                ch = js2 * ech + e + k
                gtp = aux.tile([P, C + 1], dt.float32, tag="aux")
                nc.tensor.transpose(out=gtp,
                                    in_=gs[:, (e + k) * P : (e + k + 1) * P],
                                    identity=ident[0 : C + 1, 0 : C + 1])
                rec = esb.tile([P, 1], dt.float32)
                nc.vector.reciprocal(out=rec, in_=gtp[:, C : C + 1])
                nc.vector.affine_then_add(out=oc[:, k, :], in0=gtp[:, 0:C],
                                          in1=q_sb[:, ch, 0:C],
                                          scale=rec, bias=0.0)
            ch0 = js2 * ech + e
            # the last super's stores must all ride sync: a store queued on
            # gpsimd behind its (slow, ~2.3us) teardown drain executes ~7us
            # late and stretches the measured kernel end.
            last = js2 == nsup - 1
            (nc.sync if last or (e // 2) % 2 == 0 else nc.gpsimd).dma_start(
                out=og[:, ch0 : ch0 + 2, :], in_=oc)

        for it in range(niter):
            js, m = divmod(it, nch)
            # prefetch prologue chunks: the rhs needs all chunks covering the
            # super's columns; the lhsT needs chunk m (+ lookahead).
            need_chunks(max((js + 1) * (supw // P), m + 6))
            act_full = it < act_head
            st0 = spool.tile([P, mw], dt.float32, tag="st")
            st1 = spool.tile([P, mw], dt.float32, tag="st")
            lhs = qT1[:, m * P : (m + 1) * P]
            c0 = js * supw
            nc.tensor.matmul(st0, lhs, qTt[:, c0 : c0 + mw],
                             start=True, stop=True)
            rhs2 = (qTt if act_full else qTtA)[:, c0 + mw : c0 + 2 * mw]
            nc.tensor.matmul(st1, lhs, rhs2, start=True, stop=True)
            zt = zpool.tile([P, supw], dt.int16)
            zts[it] = zt
            zb = zt.bitcast(dt.bfloat16)
            nc.scalar.activation(out=zb[:, 0:mw], in_=st0, func=AF.Exp)
            if act_full:
                nc.scalar.activation(out=zb[:, mw : 2 * mw], in_=st1,
                                     func=AF.Exp)
            else:
                nc.vector.tensor_scalar(out=zt[:, mw : 2 * mw], in0=st1,
                                        scalar1=SCH_B, scalar2=0.0,
                                        op0=ALU.add, op1=ALU.max)
            if it >= glag:
                emit_G(it - glag)
            if epi_queue and it % 2 == 0:
                emit_epi_pair(*epi_queue.pop(0))
        for it in range(niter - glag, niter):
            emit_G(it)
        while epi_queue:
            emit_epi_pair(*epi_queue.pop(0))

    nc.compile()
    return nc


_CACHE = {}


def _get_nc(**kw):
    key = tuple(sorted(kw.items()))
    if key not in _CACHE:
        _CACHE[key] = build(**kw)
    return _CACHE[key]


def kernel(x: np.ndarray, gamma: np.ndarray) -> np.ndarray:
    """Full-input entry point: x (8,16,16,16,64) f32, gamma (1,) f32."""
    if LDW_OPT:
        _patch_ldw_opt()
    from concourse.bass_utils import run_bass_kernel_spmd

    Bf, D, H, W, Cf = x.shape
    ntok = D * H * W
    xf = np.ascontiguousarray(np.asarray(x, dtype=np.float32).reshape(Bf, ntok, Cf))
    gf = np.ascontiguousarray(np.asarray(gamma, dtype=np.float32).reshape(1))
    nc = _get_nc(ntok=ntok)
    in_maps = [{"x": xf[b], "gamma": gf} for b in range(Bf)]
    res = run_bass_kernel_spmd(nc, in_maps, core_ids=list(range(Bf)))
    out = np.stack([res.results[b]["out"] for b in range(Bf)], axis=0)
    return out.reshape(x.shape).astype(x.dtype, copy=False)
